# revision 1
# baseline (speedup 1.0000x reference)
"""Trainium2 Bass kernel for nn_ButterflyNetwork — v2 (producer-ordered arena).

Design vs v1:
- v1 routed rows consumer-ordered with 92 indirect-DMA scatters (each ~1.5us
  of serialized SWDGE time on the Pool engine) -> DMA-instruction bound.
- v2 writes each module's outputs (8 z tiles + 4 act banks, one SBUF buffer
  zall [128, 12*512] f16) CONTIGUOUSLY to a producer-ordered DRAM arena with
  ONE plain HWDGE DMA, and builds the next module's input tiles with ONE
  dma_gather (1024 int16 row indices, elem 512) -> [128, 8, 512] xbig.
  2 DMA instructions per module instead of ~20.
- Init: xin pre-cast to f16 on host, uploaded, copied once into the arena's
  init region; module 0 gathers like any other module.
- Weights (host-composed rotation matrices, scale folded) as in v1, but z/C
  column order is canonical (no hot/cold reorder; dead rows land in the arena
  and are simply never gathered).
"""
import numpy as np

# ---- problem constants (hardcoded per contract) ----
COLB = 16
IN_W = 1024
OUT_W = 512
DEPTH = 8
IN_L = 4
OUT_L = 4
ACT = 8
BLOCKS = 64
CURV = 1.0
GROW = BLOCKS * ACT
TOTAL = IN_W + DEPTH * GROW  # 5120
BATCH = 4096
N_CORES = 8
BL = BATCH // N_CORES  # 512
W = BLOCKS * COLB  # 1024
NTILES = 8
NBANKS = 4
NBLK = NTILES + NBANKS          # 12 zall blocks per module
MODROWS = NBLK * 128            # 1536 arena rows per module region
ARENA_ROWS = DEPTH * MODROWS    # 12288 (no init region: module 0 reads xin)

LAST_EXEC_NS = None


# ---------------------------------------------------------------- host math
def _rotate(x, ang, stride):
    W_, B = x.shape
    xr = x.reshape(W_ // (2 * stride), 2, stride, B)
    a = ang.reshape(W_ // (2 * stride), stride)[:, :, None]
    cth, sth = np.cos(a), np.sin(a)
    lo, hi = xr[:, 0], xr[:, 1]
    return np.stack([cth * lo + sth * hi, -sth * lo + cth * hi], axis=1).reshape(W_, B)


def _module_rot_matrices(ang):
    I = np.eye(W)
    Min = I.copy()
    for l in range(IN_L):
        Min = _rotate(Min, ang[l], 2 ** (l % 4))
    Mout = I.copy()
    for l in range(OUT_L):
        Mout = _rotate(Mout, ang[IN_L + l], 2 ** ((IN_L + l) % 4))
    Min_b = np.stack([Min[16 * b:16 * b + 16, 16 * b:16 * b + 16] for b in range(BLOCKS)])
    Mout_b = np.stack([Mout[16 * b:16 * b + 16, 16 * b:16 * b + 16] for b in range(BLOCKS)])
    return Min_b, Mout_b


class _Consts:
    pass


def _build_constants(angles, biases, indices_in, scales):
    angles = np.asarray(angles, np.float64)
    biases = np.asarray(biases, np.float64)
    scales = np.asarray(scales, np.float64)
    idx = np.asarray(indices_in, np.int64)

    c = _Consts()
    c.Min, c.Mout = [], []
    for j in range(DEPTH):
        Min_b, Mout_b = _module_rot_matrices(angles[j])
        c.Min.append(Min_b)
        c.Mout.append(Mout_b)

    # first_read: which module reads each canonical row first (for scale fold)
    first_read = {}
    for j in range(DEPTH):
        for row in idx[j]:
            r = int(row)
            if r not in first_read:
                first_read[r] = j

    def src_factor(row, j):
        return scales[row] if (row < IN_W and first_read.get(int(row)) == j) else 1.0

    fac = np.ones((DEPTH, NTILES, 128))
    for j in range(DEPTH):
        for T in range(NTILES):
            for k in range(128):
                fac[j, T, k] = src_factor(idx[j][128 * T + k], j)

    # W_act: [depth, bank, half, 128k, 64m]; PSUM bank a partition q = act row 128a+q
    c.W_act = np.zeros((DEPTH, NBANKS, 2, 128, 64))
    for j in range(DEPTH):
        for a in range(NBANKS):
            for h in range(2):
                T = 2 * a + h
                for m in range(64):
                    r_act = 128 * a + 64 * h + m
                    b = r_act // ACT
                    pos = r_act % ACT
                    bloc = b - 8 * T
                    ks = 16 * bloc + np.arange(16)
                    c.W_act[j, a, h, ks, m] = c.Min[j][b][pos, :] * fac[j, T, ks]

    # C (composed in->out for nonact), D (out applied to act). Canonical column
    # order: column m of tile T = z row at position 128T+m.
    c.C = np.zeros((DEPTH - 1, NTILES, 128, 128))
    c.D = np.zeros((DEPTH - 1, NTILES, 64, 128))
    for j in range(DEPTH - 1):
        Min_b, Mout_b = c.Min[j], c.Mout[j]
        comp = np.einsum("bpk,bki->bpi", Mout_b[:, :, ACT:], Min_b[:, ACT:, :])
        for T in range(NTILES):
            for m in range(128):
                r = 128 * T + m
                b = r // 16
                pos = r % 16
                bloc = b - 8 * T
                ks = 16 * bloc + np.arange(16)
                c.C[j, T, ks, m] = comp[b][pos, :] * fac[j, T, ks]
                c.D[j, T, 8 * bloc + np.arange(ACT), m] = Mout_b[b][pos, :ACT]

    c.bias = biases.reshape(DEPTH, NBANKS, 128)

    # producer arena row of the version of canonical row `row` seen by module i
    pos_in = [dict() for _ in range(DEPTH)]
    for j in range(DEPTH):
        for p, row in enumerate(idx[j]):
            pos_in[j][int(row)] = p

    def producer_row(row, i):
        row = int(row)
        for j in range(i - 1, -1, -1):
            q = pos_in[j].get(row)
            if q is not None:  # z write of module j at position q
                return MODROWS * j + NBLK * (q % 128) + (q // 128)
            if IN_W + GROW * j <= row < IN_W + GROW * (j + 1):
                m = row - (IN_W + GROW * j)
                return MODROWS * j + NBLK * (m % 128) + (NTILES + m // 128)
        # module 0 reads all init rows (idx[0] is a full permutation), so
        # init-version rows only ever reach here with i == 0; module 0
        # gathers straight from the xin tensor.
        assert i == 0 and row < IN_W
        return row

    # gather idx tables: module i, gathered index g = 128*t + p -> consumer
    # position p' = 128*t + p; 16-row wrap [g % 16, g // 16], replicated to
    # 128 partitions (8 gpsimd cores read their own 16-row replica)
    c.gidx = np.zeros((DEPTH, 16, W // 16), np.int16)
    for i in range(DEPTH):
        for g in range(W):
            r = producer_row(idx[i][g], i)
            assert 0 <= r < (IN_W if i == 0 else ARENA_ROWS)
            c.gidx[i, g % 16, g // 16] = np.int16(r)
    c.gidx = np.tile(c.gidx, (1, 8, 1))  # [depth, 128, 64]
    return c


# ------------------------------------------------- prep/trigger sync surgery
def _fix_prep_sync(nc, prep_recs, trig_recs):
    """Tile's PREPARE_ONLY SWDGE handling leaves two holes for dma_gather
    preps: (1) data-side DMA waits (arena write completion) land on the PREP
    (forcing desc-gen back onto the critical path) instead of the trigger;
    (2) consumers of the gathered tile wait on the prep's DMASW-lane tick,
    but with an explicit completion sem the SDMA bumps that sem, not the
    DMASW lane — the lane wait is never satisfied by this transfer. Move
    (1)'s waits onto the trigger and rewrite (2)'s waits to the real sem.

    prep_recs: list of (stage, queue, mybir_name); trig_recs: list of
    (stage, queue, mybir_name) in emission order."""
    import concourse.mybir as mybir
    from concourse.tile_sem_assignment import PROC_NAME_TO_IDX

    idx_to_proc = {v: k for k, v in PROC_NAME_TO_IDX.items()}
    by_name = {}
    for f in nc.m.functions:
        for bb in f.blocks:
            for inst in bb.instructions:
                by_name[inst.name] = inst

    trig_of = {}  # (stage, queue) -> trigger inst
    for stage, q, nm in trig_recs:
        trig_of[(stage, q)] = by_name[nm]

    # lane/tick -> completion sem of that prep
    lane_ticks = {}
    prep_names = set()
    for stage, q, nm in prep_recs:
        inst = by_name[nm]
        prep_names.add(nm)
        proc = idx_to_proc.get(getattr(inst, "bass_scheduled_proc", -1), "")
        tick = getattr(inst, "bass_scheduled_tick", None)
        si = inst.sync_info
        gd = si.on_update[0]
        assert gd.ant_name.startswith("gdma"), gd
        if proc.startswith("DMASW") and tick is not None:
            lane_ticks[(proc, tick * 16)] = (gd.id, gd.ant_name)
        # (1) move DMA-lane waits from stage>=1 preps to their trigger
        if stage >= 1:
            keep, move = [], []
            for w in si.on_wait:
                (move if (w.ant_name or "").startswith(("DMAHW", "DMASW"))
                 else keep).append(w)
            if move:
                inst.sync_info = mybir.SyncInfo(
                    on_wait=keep, on_update=list(si.on_update))
                trig = trig_of[(stage, q)]
                tsi = trig.sync_info
                trig.sync_info = mybir.SyncInfo(
                    on_wait=list(tsi.on_wait) + move,
                    on_update=list(tsi.on_update))

    def remap(w):
        if not (w.ant_name or "").startswith("DMASW"):
            return w
        lane = w.ant_name.split("_")[0]
        # prep with the largest tick*16 <= wait_value on this lane
        best = None
        for (ln, v16), sem in lane_ticks.items():
            if ln == lane and v16 <= w.wait_value and (best is None or v16 > best[0]):
                best = (v16, sem)
        if best is None:
            return w
        sem_id, sem_name = best[1]
        return mybir.SyncWait(sync_type="semaphore", id=sem_id,
                              ant_name=sem_name, wait_mode="sem-ge-imm",
                              wait_value=16, wait_reg=None)

    # (2) rewrite consumer waits on DMASW lanes
    for f in nc.m.functions:
        for bb in f.blocks:
            for inst in bb.instructions:
                if inst.name in prep_names:
                    continue
                si = getattr(inst, "sync_info", None)
                if si is None or not si.on_wait:
                    continue
                new = [remap(w) for w in si.on_wait]
                if any(a is not b for a, b in zip(new, si.on_wait)):
                    inst.sync_info = mybir.SyncInfo(
                        on_wait=new, on_update=list(si.on_update))


# ------------------------------------------------- walrus sync-wait workaround
def _split_sync_waits(nc, limit=1):
    """This container's walrus build rejects >1 semaphore wait per instruction
    ("Too many sync wait commands"). Move excess waits onto NoOps placed just
    before the instruction on the same engine queue."""
    import concourse.mybir as mybir

    seq = [0]
    for f in nc.m.functions:
        for bb in f.blocks:
            insts = bb.instructions
            newlist = []
            changed = False
            for inst in insts:
                si = getattr(inst, "sync_info", None)
                waits = list(si.on_wait) if si is not None else []
                if len(waits) > limit:
                    changed = True
                    for w in waits[:-limit]:
                        nop = mybir.InstNoOp(
                            name=f"waitsplit-{seq[0]}", ins=[], outs=[])
                        seq[0] += 1
                        nop.engine = inst.engine
                        nop.sync_info = mybir.SyncInfo(on_wait=[w], on_update=[])
                        newlist.append(nop)
                    inst.sync_info = mybir.SyncInfo(
                        on_wait=waits[-limit:], on_update=list(si.on_update))
                newlist.append(inst)
            if changed:
                bb.instructions = newlist


# ---------------------------------------------------------------- bass build
def _build_bass(c, repeat=1, knobs=None):
    import concourse.bass as bass
    import concourse.mybir as mybir
    import concourse.tile as tile
    from contextlib import ExitStack

    knobs = knobs or {}
    kn_gsplit = knobs.get("gsplit", 4)   # dma_gathers per module (1, 2, or 4)
    kn_prep = knobs.get("prep", 1)       # PREPARE_ONLY + trigger gathers
    kn_prep0 = knobs.get("prep0", 1)     # use prep for stage 0 too
    kn_nq = knobs.get("nq", 4)           # SWDGE queues
    kn_qalt = knobs.get("qalt", 0)       # alternate queue pairs per stage
    f16, f32, i16 = mybir.dt.float16, mybir.dt.float32, mybir.dt.int16
    AF = mybir.ActivationFunctionType
    OP = mybir.AluOpType

    nc = bass.Bass(trn_type="TRN2", num_swdge_queues=kn_nq)
    xin = nc.dram_tensor("xin", [IN_W, BL], f16, kind="ExternalInput")
    out = nc.dram_tensor("out", [OUT_W, BL], f32, kind="ExternalOutput")

    # ---- inline constants
    wact_np = np.zeros((128, DEPTH * NBANKS * 2 * 64), np.float16)
    for j in range(DEPTH):
        for a in range(NBANKS):
            for h in range(2):
                col = ((j * NBANKS + a) * 2 + h) * 64
                wact_np[:, col:col + 64] = c.W_act[j, a, h].astype(np.float16)
    cmat_np = np.zeros((128, (DEPTH - 1) * NTILES * 128), np.float16)
    dmat_np = np.zeros((128, (DEPTH - 1) * NTILES * 128), np.float16)
    for j in range(DEPTH - 1):
        for T in range(NTILES):
            col = (j * NTILES + T) * 128
            cmat_np[:, col:col + 128] = c.C[j, T].astype(np.float16)
            po = 64 * (T % 2)
            dmat_np[po:po + 64, col:col + 128] = c.D[j, T].astype(np.float16)
    # bias columns: [j*NBANKS+a] = 0.5*bias; last col = 0.25 (Sqrt bias)
    bias_np = np.zeros((128, DEPTH * NBANKS + 1), np.float32)
    bias_np[:, DEPTH * NBANKS] = 0.25
    hbias_np = np.zeros((128, DEPTH * NBANKS), np.float32)
    for j in range(DEPTH):
        for a in range(NBANKS):
            bias_np[:, j * NBANKS + a] = c.bias[j, a].astype(np.float32)
            hbias_np[:, j * NBANKS + a] = (0.5 * c.bias[j, a]).astype(np.float32)
    gidx_np = np.zeros((128, DEPTH * (W // 16)), np.int16)
    for j in range(DEPTH):
        gidx_np[:, j * (W // 16):(j + 1) * (W // 16)] = c.gidx[j]

    wact_t = nc.inline_tensor(wact_np, name="wact")
    cmat_t = nc.inline_tensor(cmat_np, name="cmat")
    dmat_t = nc.inline_tensor(dmat_np, name="dmat")
    bias_t = nc.inline_tensor(bias_np, name="biast")
    hbias_t = nc.inline_tensor(hbias_np, name="hbiast")
    gidx_t = nc.inline_tensor(gidx_np, name="gidxt")

    from concourse import library_config

    with tile.TileContext(nc) as tc, ExitStack() as ctx:
        nc.gpsimd.load_library(library_config.mlp)
        const = ctx.enter_context(tc.tile_pool(name="const", bufs=1))
        xpool = ctx.enter_context(tc.tile_pool(name="x", bufs=2))
        zpool = ctx.enter_context(tc.tile_pool(name="z", bufs=2))
        apool = ctx.enter_context(tc.tile_pool(name="actp", bufs=2))
        pspool = ctx.enter_context(tc.tile_pool(name="ps", bufs=8, space="PSUM"))
        dram = ctx.enter_context(tc.tile_pool(name="dram", bufs=1, space="DRAM"))

        arena = dram.tile([ARENA_ROWS, BL], f16, tag="arena")

        # gidx on the scalar queue (otherwise idle at start) — it gates the
        # first gather prep; everything else streams on sync in module order
        gidx_sb = const.tile([128, gidx_np.shape[1]], i16, tag="gidx")
        nc.scalar.dma_start(out=gidx_sb[:], in_=gidx_t[:])
        wact_sbs, cmat_sbs, dmat_sbs = [], [], []
        WACT_J = NBANKS * 2 * 64   # 512 cols per module
        CMAT_J = NTILES * 128      # 1024 cols per module

        def load_consts(j):
            if j >= DEPTH or len(wact_sbs) > j:
                return
            wsb = const.tile([128, WACT_J], f16, tag=f"wact{j}")
            nc.sync.dma_start(out=wsb[:], in_=wact_t[:, j * WACT_J:(j + 1) * WACT_J])
            wact_sbs.append(wsb)
            if j < DEPTH - 1:
                csb = const.tile([128, CMAT_J], f16, tag=f"cmat{j}")
                nc.sync.dma_start(out=csb[:], in_=cmat_t[:, j * CMAT_J:(j + 1) * CMAT_J])
                cmat_sbs.append(csb)
                dsb = const.tile([128, CMAT_J], f16, tag=f"dmat{j}")
                nc.sync.dma_start(out=dsb[:], in_=dmat_t[:, j * CMAT_J:(j + 1) * CMAT_J])
                dmat_sbs.append(dsb)

        load_consts(0)
        bias_sb = const.tile([128, bias_np.shape[1]], f32, tag="bias")
        nc.sync.dma_start(out=bias_sb[:], in_=bias_t[:])
        hbias_sb = const.tile([128, hbias_np.shape[1]], f32, tag="hbias")
        nc.sync.dma_start(out=hbias_sb[:], in_=hbias_t[:])
        for _j in range(1, DEPTH):
            load_consts(_j)

        gw = W // kn_gsplit            # idxs per gather
        gt = NTILES // kn_gsplit       # tiles per gather
        gc = gw // 16                  # idx table cols per gather
        prep_recs, trig_recs = [], []

        def qn(stage_idx, h):
            if kn_qalt and kn_gsplit <= 2:
                return (stage_idx % 2) * 2 + h
            return h

        def emit_preps(stage_idx, xbig_t):
            """PREPARE_ONLY gathers for stage (rep, j): Q7 descriptor gen runs
            now (overlapped with current-stage compute); the data transfer
            fires at that stage's trigger_dma."""
            if not kn_prep:
                return
            j = stage_idx % DEPTH
            src = xin if j == 0 else arena
            for h in range(kn_gsplit):
                sem = nc.alloc_semaphore(f"gdma_{stage_idx}_{h}")
                bi = nc.gpsimd.dma_gather(
                    out_ap=xbig_t[:, h * gt:(h + 1) * gt, :],
                    in_ap=src[:],
                    idxs_ap=gidx_sb[:, j * (W // 16) + h * gc:
                                    j * (W // 16) + (h + 1) * gc],
                    num_idxs=gw,
                    num_idxs_reg=gw,
                    elem_size=BL,
                    prepare_only=True,
                    sem=sem,
                    queue_num=qn(stage_idx, h),
                )
                prep_recs.append((stage_idx, qn(stage_idx, h), bi.ins.name))

        def emit_direct_gathers(stage_idx, xbig_t):
            j = stage_idx % DEPTH
            src = xin if j == 0 else arena
            for h in range(kn_gsplit):
                nc.gpsimd.dma_gather(
                    out_ap=xbig_t[:, h * gt:(h + 1) * gt, :],
                    in_ap=src[:],
                    idxs_ap=gidx_sb[:, j * (W // 16) + h * gc:
                                    j * (W // 16) + (h + 1) * gc],
                    num_idxs=gw,
                    num_idxs_reg=gw,
                    elem_size=BL,
                    queue_num=h,
                )

        NSTAGE = repeat * DEPTH
        xbig_next = xpool.tile([128, NTILES, BL], f16, tag="xbig")
        if kn_prep and kn_prep0:
            emit_preps(0, xbig_next)

        for stage in range(NSTAGE):
            j = stage % DEPTH
            xbig = xbig_next
            if kn_prep and (stage > 0 or kn_prep0):
                # fire this stage's prepared gathers (write deps attach here)
                for h in range(kn_gsplit):
                    ti = nc.gpsimd.trigger_dma(count=None, queue_num=qn(stage, h))
                    trig_recs.append((stage, qn(stage, h), ti.ins.name))
            else:
                emit_direct_gathers(stage, xbig)
            xs = [xbig[:, T, :] for T in range(NTILES)]

            zall = zpool.tile([128, NBLK * BL], f16, tag="zall")

            # ---- act banks + activation (ao written into zall act blocks)
            aos = []
            for a in range(NBANKS):
                ps = pspool.tile([128, BL], f32, tag="ps", space="PSUM")
                for h in range(2):
                    wcol = (a * 2 + h) * 64
                    nc.tensor.matmul(
                        out=ps[64 * h:64 * h + 64, :],
                        lhsT=wact_sbs[j][:, wcol:wcol + 64],
                        rhs=xs[2 * a + h],
                        start=True, stop=True,
                        tile_position=(0, 64 * h),
                    )
                bcol = j * NBANKS + a
                # u = 0.5*(y+b); sq = u^2; v = sqrt(sq + 0.25); ao = u + v
                u = apool.tile([128, BL], f16, tag="u")
                nc.vector.tensor_scalar(out=u[:], in0=ps[:], scalar1=0.5,
                                        scalar2=hbias_sb[:, bcol:bcol + 1],
                                        op0=OP.mult, op1=OP.add)
                sq = apool.tile([128, BL], f16, tag="sq")
                v = apool.tile([128, BL], f16, tag="v")
                qcol = DEPTH * NBANKS
                nc.vector.tensor_tensor(out=sq[:], in0=u[:], in1=u[:], op=OP.mult)
                nc.scalar.activation(out=v[:], in_=sq[:], func=AF.Sqrt,
                                     bias=bias_sb[:, qcol:qcol + 1], scale=1.0)
                if j == DEPTH - 1:
                    ao = apool.tile([128, BL], f32, tag=f"aof{a}")
                    nc.vector.tensor_tensor(out=ao[:], in0=u[:], in1=v[:], op=OP.add)
                else:
                    ao = zall[:, (NTILES + a) * BL:(NTILES + a + 1) * BL]
                    nc.vector.tensor_tensor(out=ao, in0=u[:], in1=v[:], op=OP.add)
                aos.append(ao)

            if j == DEPTH - 1:
                for a in range(NBANKS):
                    nc.sync.dma_start(out=out[128 * a:128 * a + 128, :], in_=aos[a])
                if stage + 1 < NSTAGE:
                    xbig_next = xpool.tile([128, NTILES, BL], f16, tag="xbig")
                    emit_preps(stage + 1, xbig_next)
                continue

            base = MODROWS * j

            def write_blocks(t0, t1):
                # arena rows base + NBLK*p + t for t in [t0, t1)
                oap = arena[base + t0:base + t0 + NBLK * 128, :].rearrange(
                    "(p t) c -> p t c", t=NBLK)[:, 0:t1 - t0, :]
                nc.scalar.dma_start(out=oap, in_=zall[:, t0 * BL:t1 * BL])

            # act blocks (8..12) complete first — write them early
            write_blocks(NTILES, NBLK)

            # ---- z tiles: C @ x + D @ act -> zall z blocks
            for T in range(NTILES):
                col = T * 128
                po = 64 * (T % 2)
                ps = pspool.tile([128, BL], f32, tag="ps", space="PSUM")
                nc.tensor.matmul(out=ps[:], lhsT=cmat_sbs[j][:, col:col + 128],
                                 rhs=xs[T], start=True, stop=False)
                nc.tensor.matmul(out=ps[:], lhsT=dmat_sbs[j][po:po + 64, col:col + 128],
                                 rhs=aos[T // 2][po:po + 64, :],
                                 start=False, stop=True, tile_position=(po, 0))
                zslice = zall[:, T * BL:(T + 1) * BL]
                if T % 2 == 0:
                    nc.vector.tensor_copy(out=zslice, in_=ps[:])
                else:
                    nc.scalar.copy(out=zslice, in_=ps[:])
                if T == 3:
                    write_blocks(0, 4)
                elif T == 6:
                    write_blocks(4, 7)
                elif T == 7:
                    write_blocks(7, NTILES)

            # prep next stage's gathers: emitted AFTER this stage's arena
            # writes so the promoted trigger deps include them (emitting
            # earlier races the gather against the writes). The Q7 desc-gen
            # still executes early — right after this stage's triggers.
            if stage + 1 < NSTAGE:
                xbig_next = xpool.tile([128, NTILES, BL], f16, tag="xbig")
                emit_preps(stage + 1, xbig_next)

    if kn_prep:
        _fix_prep_sync(nc, prep_recs, trig_recs)
    from concourse import library_overlay
    library_overlay.lower_extended_insts(nc)
    _split_sync_waits(nc)
    return nc


# ---------------------------------------------------------------- entry point
def _prep_pjrt(nc, in_maps):
    """Build the sharded callable + device inputs; return a timed-call closure."""
    import time
    import jax
    import jax.numpy as jnp
    from jax.sharding import Mesh, PartitionSpec, NamedSharding
    from jax.experimental.shard_map import shard_map
    import concourse.mybir as mybir
    from concourse import bass2jax

    bass2jax.install_neuronx_cc_hook()
    n_cores = len(in_maps)
    partition_name = nc.partition_id_tensor.name if nc.partition_id_tensor else None
    in_names, out_names, out_avals, zero_outs = [], [], [], []
    for alloc in nc.m.functions[0].allocations:
        if not isinstance(alloc, mybir.MemoryLocationSet):
            continue
        name = alloc.memorylocations[0].name
        if alloc.kind == "ExternalInput":
            if name != partition_name:
                in_names.append(name)
        elif alloc.kind == "ExternalOutput":
            shape = tuple(alloc.tensor_shape)
            dtype = mybir.dt.np(alloc.dtype)
            out_names.append(name)
            out_avals.append(jax.core.ShapedArray(shape, dtype))
            zero_outs.append(np.zeros(shape, dtype))
    n_params = len(in_names)
    n_outs = len(out_avals)
    in_names_all = in_names + out_names + ([partition_name] if partition_name else [])
    donate = tuple(range(n_params, n_params + n_outs))

    def _body(*args):
        operands = list(args)
        if partition_name is not None:
            operands.append(bass2jax.partition_id_tensor())
        outs = bass2jax._bass_exec_p.bind(
            *operands, out_avals=tuple(out_avals), in_names=tuple(in_names_all),
            out_names=tuple(out_names), lowering_input_output_aliases=(),
            sim_require_finite=True, sim_require_nnan=True, nc=nc)
        return tuple(outs)

    devices = jax.devices()[:n_cores]
    mesh = Mesh(np.asarray(devices), ("core",))
    sharded = jax.jit(
        shard_map(_body, mesh=mesh,
                  in_specs=(PartitionSpec("core"),) * (n_params + n_outs),
                  out_specs=(PartitionSpec("core"),) * n_outs, check_rep=False),
        donate_argnums=donate, keep_unused=True)
    concat_in = [np.concatenate([np.asarray(in_maps[cix][name]) for cix in range(n_cores)], axis=0)
                 for name in in_names]
    zshapes = [((n_cores * z.shape[0],) + z.shape[1:], z.dtype) for z in zero_outs]
    shin = NamedSharding(mesh, PartitionSpec("core"))
    dev_in = [jax.device_put(x, shin) for x in concat_in]

    def call_timed():
        dev_zeros = [jax.device_put(jnp.zeros(sh, d), shin) for sh, d in zshapes]
        for z in dev_zeros:
            z.block_until_ready()
        t0 = time.perf_counter()
        outs = sharded(*dev_in, *dev_zeros)
        for o in outs:
            o.block_until_ready()
        t1 = time.perf_counter()
        return (t1 - t0) * 1e9, outs

    def results_of(outs):
        return [{name: np.asarray(outs[i]).reshape(n_cores, *out_avals[i].shape)[cix]
                 for i, name in enumerate(out_names)} for cix in range(n_cores)]

    return call_timed, results_of


def measure_pair(nc1, ncR, in_maps, iters, reps=8):
    call1, res_of = _prep_pjrt(nc1, in_maps)
    callR, _ = _prep_pjrt(ncR, in_maps)
    call1()
    callR()
    t1s, tRs = [], []
    outs = None
    for _ in range(reps):
        t1, outs = call1()
        tR, _ = callR()
        t1s.append(t1)
        tRs.append(tR)
    T = (min(tRs) - min(t1s)) / (iters - 1)
    return res_of(outs), T, min(t1s), min(tRs)


def _in_maps_of(input_data):
    x16 = np.asarray(input_data, np.float32).astype(np.float16)
    return [{"xin": np.ascontiguousarray(x16[:, i * BL:(i + 1) * BL])}
            for i in range(N_CORES)]


def measure_hw_time(input_data, scales, angles, biases, indices_in,
                    iters=16, reps=6):
    c = _build_constants(angles, biases, indices_in, scales)
    in_maps = _in_maps_of(input_data)
    nc1 = _build_bass(c)
    ncR = _build_bass(c, repeat=iters)
    res1, T, t1, tR = measure_pair(nc1, ncR, in_maps, iters, reps=max(reps, 8))
    out = np.concatenate([r["out"] for r in res1], axis=1).astype(np.float32)
    return out, T, t1, tR


def kernel(input_data, scales, angles, biases, indices_in):
    global LAST_EXEC_NS
    c = _build_constants(angles, biases, indices_in, scales)
    nc = _build_bass(c)
    in_maps = _in_maps_of(input_data)
    from concourse import bass_utils
    res = bass_utils.run_bass_kernel_spmd(
        nc, in_maps, core_ids=list(range(N_CORES)), trace=False,
    )
    LAST_EXEC_NS = res.exec_time_ns
    out = np.concatenate([r["out"] for r in res.results], axis=1)
    return out.astype(np.float32)

